# revision 16
# baseline (speedup 1.0000x reference)
"""Trainium2 Bass kernel for an MLP flow-matching GNN (message passing).

Strategy (8 NeuronCores, SPMD):
  - Host: sort edges by destination, partition nodes into 8 contiguous ranges
    (one per core), pad each core's node count to a multiple of 128.  Nodes are
    processed in UNIFORM groups of 128; each group's incident edges are padded
    to a uniform per-group edge capacity (EG = PPG*128, PPG = max group degree
    sum over all cores, in panels of 128).  This makes every slice bound
    core-uniform so a single SPMD program works for all cores.
  - The edge-MLP first layer is pre-projected into per-node tables:
        h1d = h @ W1a + x_t @ W1c + b1      (dst table, local per core)
        h1s = h @ W1b - x_t @ W1c           (src table, AllGathered)
    so that m1[e] = h1d[dst] + h1s[src] includes the relative-position term
    (rel @ W1c = Y[dst] - Y[src]) and bias with no per-edge matmul.
  - Device, per layer:
      edge phase:  two batched indirect-DMA gathers per chunk of groups (dst
                   rows, then src rows accumulated via the DMA CCE add) ->
                   SiLU -> PE transpose -> edge-MLP2 matmul -> +b2, SiLU ->
                   segment-sum via matmul against an is_equal indicator,
                   accumulated in PSUM across the whole 128-node group ->
                   one contiguous store per group.
      node phase:  DMA-transpose loads of h and agg (feature-major), node MLP
                   + residual + chunk-local LayerNorm, and the next layer's
                   projected tables (with the x_t @ W1c fold done by a K=4
                   accumulate matmul).
      comm:        one AllGather of the src table only.
  - Host: final projection + MSE loss (tiny).
"""

import numpy as np
import ml_dtypes

BF16 = ml_dtypes.bfloat16
EPS = 1e-5
NCORES = 8
P = 128          # partition width / hidden size (H must equal 128)
GCH = 7          # groups gathered per indirect-DMA instruction pair
CW = 4           # node windows per node-phase chunk


def _silu(x):
    return x * (1.0 / (1.0 + np.exp(-x)))


# ----------------------------------------------------------------------------
# Host-side preprocessing
# ----------------------------------------------------------------------------

def _preprocess(inputs):
    pos0 = np.asarray(inputs["pos0"], np.float32)
    pos1 = np.asarray(inputs["pos1"], np.float32)
    z = np.asarray(inputs["z"], np.float32)
    t = np.asarray(inputs["t"], np.float32)
    edge_index = np.asarray(inputs["edge_index"])
    batch = np.asarray(inputs["batch"])
    ew1 = np.asarray(inputs["ew1"], np.float32)
    eb1 = np.asarray(inputs["eb1"], np.float32)
    ew2 = np.asarray(inputs["ew2"], np.float32)
    eb2 = np.asarray(inputs["eb2"], np.float32)
    nw1 = np.asarray(inputs["nw1"], np.float32)
    nb1 = np.asarray(inputs["nb1"], np.float32)
    nw2 = np.asarray(inputs["nw2"], np.float32)
    nb2 = np.asarray(inputs["nb2"], np.float32)
    ln_g = np.asarray(inputs["ln_g"], np.float32)
    ln_b = np.asarray(inputs["ln_b"], np.float32)

    V = pos0.shape[0]
    L = ew1.shape[0]
    H = ew1.shape[2]
    assert H == P
    nv = V // NCORES
    assert nv * NCORES == V
    ngrp = (nv + P - 1) // P
    nvp = ngrp * P

    ts = float(t[0])
    x_t = (1.0 - ts) * pos0 + ts * pos1
    target = pos1 - pos0

    te_w1 = np.asarray(inputs["te_w1"], np.float32)
    te_b1 = np.asarray(inputs["te_b1"], np.float32)
    te_w2 = np.asarray(inputs["te_w2"], np.float32)
    te_b2 = np.asarray(inputs["te_b2"], np.float32)
    cp_w = np.asarray(inputs["cp_w"], np.float32)
    cp_b = np.asarray(inputs["cp_b"], np.float32)

    t_emb = _silu(np.array([[ts]], np.float32) @ te_w1 + te_b1) @ te_w2 + te_b2
    h0 = np.concatenate(
        [z[batch], np.broadcast_to(t_emb, (V, t_emb.shape[1]))], axis=1
    ) @ cp_w + cp_b  # [V, H] f32

    # layer-0 folded tables (rel-pos + bias folded in; see module docstring)
    Y = x_t @ ew1[0, 2 * H:2 * H + 3]                 # [V, H]
    h1d0 = h0 @ ew1[0, :H] + Y + eb1[0]
    h1s0 = h0 @ ew1[0, H:2 * H] - Y

    def pad_rows(a):
        out = np.zeros((nvp, a.shape[1]), a.dtype)
        out[:a.shape[0]] = a
        return out

    h1s_full0 = np.concatenate(
        [pad_rows(h1s0[c * nv:(c + 1) * nv]) for c in range(NCORES)], axis=0
    ).astype(BF16)                                    # [8*nvp, H]

    # edges sorted by destination, split at core boundaries
    src_g = edge_index[0].astype(np.int64)
    dst_g = edge_index[1].astype(np.int64)
    order = np.argsort(dst_g, kind="stable")
    dst_s = dst_g[order]
    src_s = src_g[order]
    bounds = np.searchsorted(dst_s, np.arange(0, V + 1, nv))

    # per-group edge capacity: uniform across cores, variable across groups
    per_edges = []
    maxcnt = np.zeros(ngrp, np.int64)
    for c in range(NCORES):
        e0, e1 = int(bounds[c]), int(bounds[c + 1])
        ld = (dst_s[e0:e1] - c * nv).astype(np.int64)
        sg = src_s[e0:e1]
        grp = ld // P
        cnt = np.bincount(grp, minlength=ngrp)
        maxcnt = np.maximum(maxcnt, cnt)
        per_edges.append((ld, sg, cnt))
    ppg = ((maxcnt + P - 1) // P).astype(np.int64)
    coff = np.concatenate([[0], np.cumsum(ppg)])  # per-group column offsets
    ncol = int(coff[-1])

    per_core = []
    for c in range(NCORES):
        ld, sg, cnt = per_edges[c]
        srcidx = np.zeros(ncol * P, np.int32)
        dstrel = np.full(ncol * P, -1, np.int32)
        starts = np.concatenate([[0], np.cumsum(cnt)])
        for g in range(ngrp):
            s0, s1 = int(starts[g]), int(starts[g + 1])
            n = s1 - s0
            sl = slice(int(coff[g]) * P, int(coff[g]) * P + n)
            srow = (sg[s0:s1] // nv) * nvp + (sg[s0:s1] % nv)
            srcidx[sl] = srow
            dstrel[sl] = ld[s0:s1] - g * P
        # slot (j, p) = edge j*128 + p  ->  column-major [P, ncol]
        per_core.append({
            "srcidx": srcidx.reshape(ncol, P).T.copy(),
            "dstrel": dstrel.reshape(ncol, P).T.copy(),
            "dstrelF": dstrel.astype(BF16)[None, :].copy(),
            "h1d_own0": pad_rows(h1d0[c * nv:(c + 1) * nv]).astype(BF16),
            "h0own": pad_rows(h0[c * nv:(c + 1) * nv]).astype(BF16),
            "x4own": np.concatenate(
                [pad_rows(x_t[c * nv:(c + 1) * nv]).T,
                 np.ones((1, nvp), np.float32)], axis=0),   # [4, nvp]
        })

    # device weight layouts (concat layers along free dim)
    cat = lambda m: np.concatenate([m[l] for l in range(L)], axis=1)
    bc = lambda v: np.concatenate(
        [np.broadcast_to(v[l], (P, H)) for l in range(L)], axis=1)
    w1a = cat(ew1[:, :H]).astype(BF16)
    w1b = cat(ew1[:, H:2 * H]).astype(BF16)
    w2 = cat(ew2).astype(BF16)
    w1c4d = np.concatenate(
        [np.concatenate([ew1[l, 2 * H:], eb1[l][None, :]], 0)
         for l in range(L)], axis=1).astype(np.float32)        # [4, L*H]
    w1c4s = np.concatenate(
        [np.concatenate([-ew1[l, 2 * H:], np.zeros((1, H), np.float32)], 0)
         for l in range(L)], axis=1).astype(np.float32)
    weights = dict(
        w1a=w1a, w1b=w1b, w2=w2, w1c4d=w1c4d, w1c4s=w1c4s,
        b2bc=bc(eb2).astype(np.float32),
        nw1h=cat(nw1[:, :H]).astype(BF16),
        nw1a=cat(nw1[:, H:]).astype(BF16),
        nw2=cat(nw2).astype(BF16),
        nb1c=nb1.T.astype(np.float32).copy(),                  # [H, L]
        nb2bc=bc(nb2).astype(np.float32),
        lngbc=bc(ln_g).astype(np.float32),
        lnbbc=bc(ln_b).astype(np.float32),
        ident=np.eye(P, dtype=BF16),
        iota=np.tile(np.arange(P, dtype=np.int32), (P, 1)),
        iotacolf=np.arange(P, dtype=np.float32)[:, None].copy(),
    )

    geom = dict(V=V, L=L, H=H, nv=nv, nvp=nvp, ngrp=ngrp,
                ppg=[int(x) for x in ppg], ncol=ncol)
    host = dict(h1s_full0=h1s_full0, target=target,
                op_w=np.asarray(inputs["op_w"], np.float32),
                op_b=np.asarray(inputs["op_b"], np.float32))
    return geom, per_core, weights, host


# ----------------------------------------------------------------------------
# Device program
# ----------------------------------------------------------------------------

SILU_DECOMPOSED = False  # sim has no Silu table; set True for CoreSim runs


def _build_program(geom):
    import concourse.bass as bass
    import concourse.bacc as bacc
    import concourse.mybir as mybir
    import concourse.tile as tile

    dt = mybir.dt
    AF = mybir.ActivationFunctionType
    ALU = mybir.AluOpType
    IOA = bass.IndirectOffsetOnAxis
    AX = mybir.AxisListType

    L, nvp, ngrp = geom["L"], geom["nvp"], geom["ngrp"]
    ppg, ncol = geom["ppg"], geom["ncol"]
    coff = [0]
    for x in ppg:
        coff.append(coff[-1] + x)
    maxppg = max(ppg)
    NW = ngrp  # node windows per core

    nc = bacc.Bacc(num_devices=NCORES)

    # ---- I/O ----
    h1sf0 = nc.declare_dram_parameter("h1s_full0", [NCORES * nvp, P],
                                      dt.bfloat16, isOutput=False)
    h1d0_d = nc.declare_dram_parameter("h1d_own0", [nvp, P], dt.bfloat16,
                                       isOutput=False)
    h0own = nc.declare_dram_parameter("h0own", [nvp, P], dt.bfloat16,
                                      isOutput=False)
    srcidx_d = nc.declare_dram_parameter("srcidx", [P, ncol], dt.int32, isOutput=False)
    dstrel_d = nc.declare_dram_parameter("dstrel", [P, ncol], dt.int32, isOutput=False)
    dstrelF_d = nc.declare_dram_parameter("dstrelF", [1, ncol * P], dt.bfloat16, isOutput=False)
    x4own_d = nc.declare_dram_parameter("x4own", [4, nvp], dt.float32, isOutput=False)
    w1a_d = nc.declare_dram_parameter("w1a", [P, L * P], dt.bfloat16, isOutput=False)
    w1b_d = nc.declare_dram_parameter("w1b", [P, L * P], dt.bfloat16, isOutput=False)
    w2_d = nc.declare_dram_parameter("w2", [P, L * P], dt.bfloat16, isOutput=False)
    w1c4d_d = nc.declare_dram_parameter("w1c4d", [4, L * P], dt.float32, isOutput=False)
    w1c4s_d = nc.declare_dram_parameter("w1c4s", [4, L * P], dt.float32, isOutput=False)
    b2bc_d = nc.declare_dram_parameter("b2bc", [P, L * P], dt.float32, isOutput=False)
    nw1h_d = nc.declare_dram_parameter("nw1h", [P, L * P], dt.bfloat16, isOutput=False)
    nw1a_d = nc.declare_dram_parameter("nw1a", [P, L * P], dt.bfloat16, isOutput=False)
    nw2_d = nc.declare_dram_parameter("nw2", [P, L * P], dt.bfloat16, isOutput=False)
    nb1c_d = nc.declare_dram_parameter("nb1c", [P, L], dt.float32, isOutput=False)
    nb2bc_d = nc.declare_dram_parameter("nb2bc", [P, L * P], dt.float32, isOutput=False)
    lngbc_d = nc.declare_dram_parameter("lngbc", [P, L * P], dt.float32, isOutput=False)
    lnbbc_d = nc.declare_dram_parameter("lnbbc", [P, L * P], dt.float32, isOutput=False)
    ident_d = nc.declare_dram_parameter("ident", [P, P], dt.bfloat16, isOutput=False)
    iota_d = nc.declare_dram_parameter("iota", [P, P], dt.int32, isOutput=False)
    iotacolf_d = nc.declare_dram_parameter("iotacolf", [P, 1], dt.float32, isOutput=False)
    hout_d = nc.declare_dram_parameter("hout", [nvp, P], dt.float32, isOutput=True)

    # ---- internal DRAM ----
    agg_hbm = [nc.dram_tensor(f"agg_hbm{i}", [nvp, P], dt.bfloat16)
               for i in range(2)]
    hown = [nc.dram_tensor(f"hown{l + 1}", [nvp, P], dt.bfloat16)
            for l in range(L - 1)]
    h1down = [nc.dram_tensor(f"h1down{l + 1}", [nvp, P], dt.bfloat16)
              for l in range(L - 1)]
    h1sown = [nc.dram_tensor(f"h1sown{l + 1}", [nvp, P], dt.bfloat16)
              for l in range(L - 1)]
    h1sfull = [nc.dram_tensor(f"h1sfull{l + 1}", [NCORES * nvp, P],
                              dt.bfloat16, addr_space="Shared")
               for l in range(L - 1)]

    groups = [list(range(NCORES))]

    with tile.TileContext(nc) as tc:
        with (
            tc.tile_pool(name="const", bufs=1) as cpool,
            tc.tile_pool(name="gather", bufs=2) as gpool,
            tc.tile_pool(name="gsrc", bufs=4) as gspool,
            tc.tile_pool(name="work", bufs=3) as wpool,
            tc.tile_pool(name="small", bufs=4) as spool,
            tc.tile_pool(name="pmB", bufs=6, space="PSUM") as pmB,
            tc.tile_pool(name="pagg", bufs=2, space="PSUM") as pagg,
        ):
            def cload(src, shape, dtype, tag):
                t_ = cpool.tile(shape, dtype, tag=tag)
                nc.sync.dma_start(out=t_[:], in_=src[:, :])
                return t_

            identsb = cload(ident_d, [P, P], dt.bfloat16, "ident")
            iotasb = cload(iota_d, [P, P], dt.int32, "iota")
            onesb = cpool.tile([1, P], dt.bfloat16, tag="ones")
            nc.vector.memset(onesb[:], 1.0)
            iotacol = cload(iotacolf_d, [P, 1], dt.float32, "iotacol")
            srcidxsb = cload(srcidx_d, [P, ncol], dt.int32, "srcidx")
            dstrelsb = cload(dstrel_d, [P, ncol], dt.int32, "dstrel")
            x4sb = cload(x4own_d, [4, nvp], dt.float32, "x4own")
            w1asb = cload(w1a_d, [P, L * P], dt.bfloat16, "w1a")
            w1bsb = cload(w1b_d, [P, L * P], dt.bfloat16, "w1b")
            w2sb = cload(w2_d, [P, L * P], dt.bfloat16, "w2")
            w1c4dsb = cload(w1c4d_d, [4, L * P], dt.float32, "w1c4d")
            w1c4ssb = cload(w1c4s_d, [4, L * P], dt.float32, "w1c4s")
            b2bcsb = cload(b2bc_d, [P, L * P], dt.float32, "b2bc")
            nw1hsb = cload(nw1h_d, [P, L * P], dt.bfloat16, "nw1h")
            nw1asb = cload(nw1a_d, [P, L * P], dt.bfloat16, "nw1a")
            nw2sb = cload(nw2_d, [P, L * P], dt.bfloat16, "nw2")
            nb1csb = cload(nb1c_d, [P, L], dt.float32, "nb1c")
            nb2bcsb = cload(nb2bc_d, [P, L * P], dt.float32, "nb2bc")
            lngbcsb = cload(lngbc_d, [P, L * P], dt.float32, "lngbc")
            lnbbcsb = cload(lnbbc_d, [P, L * P], dt.float32, "lnbbc")

            def emit_silu(out_ap, in_ap, scratch_pool, tag, bias=0.0):
                if not SILU_DECOMPOSED:
                    nc.scalar.activation(out_ap, in_ap, AF.Silu, bias=bias)
                else:
                    sg = scratch_pool.tile(
                        [P, in_ap.shape[-1] if in_ap.ndim == 2 else P],
                        dt.float32, tag=tag)
                    sga = sg[:in_ap.shape[0], :in_ap.shape[-1]]
                    nc.scalar.activation(sga, in_ap, AF.Sigmoid, bias=bias)
                    nc.vector.tensor_tensor(out=out_ap, in0=in_ap, in1=sga,
                                            op=ALU.mult)

            def edge_chunk(l, g0, g1, h1d_dram, h1s_dram, agg_hbm):
                lsl = slice(l * P, (l + 1) * P)
                if True:
                    nrows = (g1 - g0) * P
                    ck0, ck1 = coff[g0], coff[g1]
                    # dst node rows for these groups (contiguous, node-major)
                    nodes = gpool.tile([P, GCH * P], dt.bfloat16, tag="nodes")
                    nc.sync.dma_start(
                        out=nodes[:, :nrows].rearrange("p (g h) -> p g h", h=P),
                        in_=h1d_dram[g0 * P:g1 * P, :].rearrange(
                            "(g p) h -> p g h", p=P))
                    # per-edge slot ids along the free axis (for S_T build)
                    relf = gpool.tile([1, GCH * maxppg * P], dt.bfloat16,
                                      tag="relf")
                    nc.sync.dma_start(
                        out=relf[:, :(ck1 - ck0) * P],
                        in_=dstrelF_d[:, ck0 * P:ck1 * P])
                    for g in range(g0, g1):
                        aggp = pagg.tile([P, P], dt.float32, tag="agg")
                        nodes_g = nodes[:, (g - g0) * P:(g - g0 + 1) * P]
                        # src rows: burst all of this group's panel gathers
                        # into one tile so the gpsimd stream runs uninterrupted
                        gsg = gspool.tile([P, maxppg * P], dt.bfloat16,
                                          tag="gs")
                        for j in range(ppg[g]):
                            col = coff[g] + j
                            nc.gpsimd.indirect_dma_start(
                                out=gsg[:, j * P:(j + 1) * P],
                                out_offset=None,
                                in_=h1s_dram[:, :],
                                in_offset=IOA(ap=srcidxsb[:, col:col + 1],
                                              axis=0))
                        for js in range(0, ppg[g], 4):
                            je = min(js + 4, ppg[g])
                            nsb = je - js
                            gs = gsg[:, js * P:je * P]
                            # S_T[slot, e] via ones-matmul broadcast + is_equal
                            rsl = slice((coff[g] - ck0 + js) * P,
                                        (coff[g] - ck0 + je) * P)
                            bcp = pmB.tile([P, 4 * P], dt.float32, tag="B")
                            nc.tensor.matmul(bcp[:, :nsb * P], lhsT=onesb[:],
                                             rhs=relf[:, rsl],
                                             start=True, stop=True)
                            ST = wpool.tile([P, 4 * P], dt.bfloat16, tag="ST")
                            nc.vector.tensor_scalar(
                                ST[:, :nsb * P], bcp[:, :nsb * P],
                                iotacol[:], None, op0=ALU.is_equal)
                            # m1t (feature-major, PSUM): gathered src rows are
                            # transposed in via matmul-with-identity, the dst
                            # expansion streams S_T against the node tile
                            m1tp = pmB.tile([P, 4 * P], dt.float32, tag="B")
                            for k in range(nsb):
                                ksl = slice(k * P, (k + 1) * P)
                                nc.tensor.matmul(
                                    m1tp[:, ksl], lhsT=gs[:, ksl],
                                    rhs=identsb[:], start=True, stop=False,
                                    skip_group_check=True)
                                nc.tensor.matmul(
                                    m1tp[:, ksl], lhsT=nodes_g,
                                    rhs=ST[:, ksl], start=False, stop=True,
                                    skip_group_check=True)
                            m1t = wpool.tile([P, 4 * P], dt.bfloat16, tag="m1tsb")
                            emit_silu(m1t[:, :nsb * P], m1tp[:, :nsb * P],
                                      wpool, "sg1")
                            m2p = pmB.tile([P, 4 * P], dt.float32, tag="B")
                            for k in range(nsb):
                                nc.tensor.matmul(
                                    m2p[:, k * P:(k + 1) * P],
                                    lhsT=m1t[:, k * P:(k + 1) * P],
                                    rhs=w2sb[:, lsl], start=True, stop=True)
                            nc.vector.tensor_tensor(
                                out=m2p[:, :nsb * P].rearrange(
                                    "p (j h) -> p j h", h=P),
                                in0=m2p[:, :nsb * P].rearrange(
                                    "p (j h) -> p j h", h=P),
                                in1=b2bcsb[:, lsl].unsqueeze(1)
                                    .to_broadcast([P, nsb, P]),
                                op=ALU.add)
                            m2s = wpool.tile([P, 4 * P], dt.bfloat16, tag="m2s")
                            emit_silu(m2s[:, :nsb * P], m2p[:, :nsb * P],
                                      wpool, "sg2")
                            S = wpool.tile([P, 4 * P], dt.bfloat16, tag="S")
                            csl = slice(coff[g] + js, coff[g] + je)
                            nc.vector.tensor_tensor(
                                out=S[:, :nsb * P].rearrange(
                                    "p (j s) -> p j s", s=P),
                                in0=dstrelsb[:, csl].unsqueeze(2)
                                    .to_broadcast([P, nsb, P]),
                                in1=iotasb[:].unsqueeze(1)
                                    .to_broadcast([P, nsb, P]),
                                op=ALU.is_equal)
                            for k in range(nsb):
                                nc.tensor.matmul(
                                    aggp[:], lhsT=S[:, k * P:(k + 1) * P],
                                    rhs=m2s[:, k * P:(k + 1) * P],
                                    start=(js == 0 and k == 0),
                                    stop=(je == ppg[g] and k == nsb - 1),
                                    skip_group_check=True)
                        aggsb = spool.tile([P, P], dt.bfloat16, tag="aggsb")
                        nc.scalar.activation(aggsb[:], aggp[:], AF.Copy)
                        nc.sync.dma_start(out=agg_hbm[g * P:(g + 1) * P, :],
                                          in_=aggsb[:])

            def node_chunk(l, w0, w1, hprev, hnext, h1d_next, h1s_next,
                           agg_hbm):
                lsl = slice(l * P, (l + 1) * P)
                last = l == L - 1
                if True:
                    nw = w1 - w0
                    cnt = nw * P
                    rows = slice(w0 * P, w0 * P + cnt)
                    hwt = spool.tile([P, CW * P], dt.bfloat16, tag="hwt")
                    nc.sync.dma_start_transpose(hwt[:, :cnt], hprev[rows, :])
                    awt = spool.tile([P, CW * P], dt.bfloat16, tag="awt")
                    nc.sync.dma_start_transpose(awt[:, :cnt], agg_hbm[rows, :])
                    hwin = spool.tile([P, CW * P], dt.bfloat16, tag="hwin")
                    nc.sync.dma_start(
                        out=hwin[:, :cnt].rearrange("p (w h) -> p w h", h=P),
                        in_=hprev[rows, :].rearrange("(w p) h -> p w h", p=P))

                    n1p = pmB.tile([P, CW * P], dt.float32, tag="B")
                    nc.tensor.matmul(n1p[:, :cnt], lhsT=nw1hsb[:, lsl],
                                     rhs=hwt[:, :cnt], start=True, stop=False)
                    nc.tensor.matmul(n1p[:, :cnt], lhsT=nw1asb[:, lsl],
                                     rhs=awt[:, :cnt], start=False, stop=True)
                    n1s = spool.tile([P, CW * P], dt.bfloat16, tag="n1s")
                    emit_silu(n1s[:, :cnt], n1p[:, :cnt], spool, "sgn",
                              bias=nb1csb[:, l:l + 1])

                    n2p = pmB.tile([P, CW * P], dt.float32, tag="B")
                    for w in range(nw):
                        nc.tensor.matmul(n2p[:, w * P:(w + 1) * P],
                                         lhsT=n1s[:, w * P:(w + 1) * P],
                                         rhs=nw2sb[:, lsl],
                                         start=True, stop=True)
                    nc.vector.tensor_tensor(
                        out=n2p[:, :cnt].rearrange("p (w h) -> p w h", h=P),
                        in0=n2p[:, :cnt].rearrange("p (w h) -> p w h", h=P),
                        in1=nb2bcsb[:, lsl].unsqueeze(1)
                            .to_broadcast([P, nw, P]),
                        op=ALU.add)
                    nc.vector.tensor_tensor(
                        out=n2p[:, :cnt].rearrange("p (w h) -> p w h", h=P),
                        in0=n2p[:, :cnt].rearrange("p (w h) -> p w h", h=P),
                        in1=hwin[:, :cnt].rearrange("p (w h) -> p w h", h=P),
                        op=ALU.add)

                    mu = spool.tile([P, CW], dt.float32, tag="mu")
                    var = spool.tile([P, CW], dt.float32, tag="var")
                    sq = spool.tile([P, P], dt.float32, tag="sq")
                    for w in range(nw):
                        wsl = slice(w * P, (w + 1) * P)
                        nc.vector.reduce_sum(mu[:, w:w + 1], n2p[:, wsl],
                                             axis=AX.X)
                    nc.vector.tensor_scalar_mul(mu[:, :nw], mu[:, :nw], 1.0 / P)
                    for w in range(nw):
                        wsl = slice(w * P, (w + 1) * P)
                        nc.vector.tensor_scalar_sub(n2p[:, wsl], n2p[:, wsl],
                                                    mu[:, w:w + 1])
                        nc.scalar.activation(sq[:], n2p[:, wsl], AF.Square,
                                             accum_out=var[:, w:w + 1])
                    nc.vector.tensor_scalar(var[:, :nw], var[:, :nw],
                                            1.0 / P, EPS,
                                            op0=ALU.mult, op1=ALU.add)
                    srt = spool.tile([P, CW], dt.float32, tag="srt")
                    nc.scalar.activation(srt[:, :nw], var[:, :nw], AF.Sqrt)
                    rstd = spool.tile([P, CW], dt.float32, tag="rstd")
                    nc.vector.reciprocal(rstd[:, :nw], srt[:, :nw])

                    xn = spool.tile([P, CW * P], dt.float32, tag="xn")
                    for w in range(nw):
                        wsl = slice(w * P, (w + 1) * P)
                        nc.vector.tensor_scalar_mul(xn[:, wsl], n2p[:, wsl],
                                                    rstd[:, w:w + 1])
                    nc.vector.tensor_tensor(
                        out=xn[:, :cnt].rearrange("p (w h) -> p w h", h=P),
                        in0=xn[:, :cnt].rearrange("p (w h) -> p w h", h=P),
                        in1=lngbcsb[:, lsl].unsqueeze(1)
                            .to_broadcast([P, nw, P]),
                        op=ALU.mult)
                    nc.vector.tensor_tensor(
                        out=xn[:, :cnt].rearrange("p (w h) -> p w h", h=P),
                        in0=xn[:, :cnt].rearrange("p (w h) -> p w h", h=P),
                        in1=lnbbcsb[:, lsl].unsqueeze(1)
                            .to_broadcast([P, nw, P]),
                        op=ALU.add)

                    if last:
                        nc.sync.dma_start(
                            out=hout_d[rows, :].rearrange(
                                "(w p) h -> p w h", p=P),
                            in_=xn[:, :cnt].rearrange("p (w h) -> p w h", h=P))
                        return

                    hnb = spool.tile([P, CW * P], dt.bfloat16, tag="hnb")
                    nc.vector.tensor_copy(hnb[:, :cnt], xn[:, :cnt])
                    nc.sync.dma_start(
                        out=hnext[rows, :].rearrange("(w p) h -> p w h", p=P),
                        in_=hnb[:, :cnt].rearrange("p (w h) -> p w h", h=P))
                    hnt = spool.tile([P, CW * P], dt.bfloat16, tag="hnt")
                    nc.sync.dma_start_transpose(hnt[:, :cnt], hnext[rows, :])

                    nsl = slice((l + 1) * P, (l + 2) * P)
                    for (tbl, wab, w1c4b, tag) in (
                        (h1d_next, w1asb, w1c4dsb, "pd"),
                        (h1s_next, w1bsb, w1c4ssb, "ps"),
                    ):
                        pp = pmB.tile([P, CW * P], dt.float32, tag="B")
                        for w in range(nw):
                            wsl = slice(w * P, (w + 1) * P)
                            nc.tensor.matmul(pp[:, wsl],
                                             lhsT=hnt[:, wsl],
                                             rhs=wab[:, nsl],
                                             start=True, stop=False)
                            nc.tensor.matmul(
                                pp[:, wsl],
                                lhsT=x4sb[:, w0 * P + w * P:
                                          w0 * P + (w + 1) * P],
                                rhs=w1c4b[:, nsl],
                                start=False, stop=True)
                        pb = spool.tile([P, CW * P], dt.bfloat16, tag=tag)
                        nc.scalar.activation(pb[:, :cnt], pp[:, :cnt], AF.Copy)
                        nc.sync.dma_start(
                            out=tbl[rows, :].rearrange("(w p) h -> p w h", p=P),
                            in_=pb[:, :cnt].rearrange("p (w h) -> p w h", h=P))

            for l in range(L):
                h1d_dram = h1d0_d if l == 0 else h1down[l - 1]
                h1s_dram = h1sf0 if l == 0 else h1sfull[l - 1]
                hprev = h0own if l == 0 else hown[l - 1]
                agg = agg_hbm[l % 2]
                nxt = (hown[l], h1down[l], h1sown[l]) if l < L - 1 else \
                    (None, None, None)
                for g0 in range(0, ngrp, GCH):
                    g1 = min(g0 + GCH, ngrp)
                    edge_chunk(l, g0, g1, h1d_dram, h1s_dram, agg)
                    for w0 in range(g0, g1, CW):
                        w1 = min(w0 + CW, g1)
                        node_chunk(l, w0, w1, hprev, *nxt, agg)
                if l < L - 1:
                    nc.gpsimd.collective_compute(
                        "AllGather", mybir.AluOpType.bypass,
                        replica_groups=groups,
                        ins=[h1sown[l][:, :]], outs=[h1sfull[l][:, :]])

    nc.finalize()
    return nc


# ----------------------------------------------------------------------------
# Entry point
# ----------------------------------------------------------------------------

def _make_in_maps(geom, per_core, weights, host):
    in_maps = []
    for c in range(NCORES):
        pc = per_core[c]
        m = {
            "h1s_full0": host["h1s_full0"],
            "h1d_own0": pc["h1d_own0"],
            "h0own": pc["h0own"],
            "srcidx": pc["srcidx"], "dstrel": pc["dstrel"],
            "dstrelF": pc["dstrelF"], "x4own": pc["x4own"],
        }
        m.update(weights)
        in_maps.append(m)
    return in_maps


def _postprocess(geom, host, houts):
    nv = geom["nv"]
    h = np.concatenate([ho[:nv] for ho in houts], axis=0).astype(np.float32)
    v_pred = h @ host["op_w"] + host["op_b"]
    diff = v_pred - host["target"]
    return np.float32(np.mean(diff.astype(np.float64) ** 2))


def kernel(**inputs):
    from concourse.bass_utils import run_bass_kernel_spmd

    geom, per_core, weights, host = _preprocess(inputs)
    nc = _build_program(geom)
    in_maps = _make_in_maps(geom, per_core, weights, host)
    res = run_bass_kernel_spmd(nc, in_maps, list(range(NCORES)))
    houts = [res.results[c]["hout"] for c in range(NCORES)]
    return _postprocess(geom, host, houts)
